# revision 11
# baseline (speedup 1.0000x reference)
"""Trainium2 Bass kernel for nn_Projector (dense_cnn).

Pipeline per sample:
  up2(x) -> conv1 3x3 512->512 + BN + ReLU -> up2 -> conv2 3x3 512->256 + BN +
  ReLU -> conv3 1x1 + bias -> dynamic per-sample 3x3 conv (nq query filters
  collapsed by linearity into a single filter + bias) -> scalar output map.

Strategy: pure data parallel over batch (16 samples -> 8 cores x 2).
All convs run on the PE as f32r (TF32-class) shift-accumulate matmuls with
channels on partitions and spatial pixels in the free dim.  The bilinear
2x upsample (exact jax.image.resize semantics incl. edge clamp) runs on the
DVE as 2-tap blends; its 0.75 factor per direction is folded into the conv
weights (x0.5625), so interior blends are 1 STT op and edges are a 4/3 scale.
BN is folded into conv weights/bias on the host.  conv2+conv3+dynamic conv
are fused in a 26-band loop over output rows with a sliding h4 window, so no
intermediate ever touches DRAM.
"""
import numpy as np

import concourse.bass as bass
import concourse.bacc as bacc
import concourse.mybir as mybir
import concourse.tile as tile
from concourse.bass_utils import run_bass_kernel_spmd

dt = mybir.dt
AF = mybir.ActivationFunctionType
AL = mybir.AluOpType
F32 = dt.float32
F32R = dt.float32r

N_CORES = 8
SPC = 2  # samples per core
EPS = 1e-5
NQ = 12
THIRD = 1.0 / 3.0
EDGE = 4.0 / 3.0

# conv1 output row blocks (start, rows)
BLOCKS1 = [(0, 9), (9, 9), (18, 9), (27, 9), (36, 9), (45, 7)]
NB2 = 26  # conv2/dyn bands of 4 rows

_CACHE = {}


def _rowblend(nc, src3, dst3, r_lo, r_hi, hin):
    """Blend up2 rows r in [r_lo, r_hi) (valid rows only, 0<=r<2*hin) from
    src3 (128, hin, W) into dst3 slots [r - r_lo].  Unnormalized by 1/0.75."""
    ev = [r for r in range(r_lo, r_hi) if r % 2 == 0 and r >= 2]
    if ev:
        k0 = ev[0] // 2
        n = len(ev)
        i0 = ev[0] - r_lo
        nc.vector.scalar_tensor_tensor(
            dst3[:, i0:i0 + 2 * (n - 1) + 1:2, :],
            src3[:, k0 - 1:k0 - 1 + n, :], THIRD, src3[:, k0:k0 + n, :],
            AL.mult, AL.add)
    od = [r for r in range(r_lo, r_hi) if r % 2 == 1 and r <= 2 * hin - 3]
    if od:
        k0 = (od[0] - 1) // 2
        n = len(od)
        i0 = od[0] - r_lo
        nc.vector.scalar_tensor_tensor(
            dst3[:, i0:i0 + 2 * (n - 1) + 1:2, :],
            src3[:, k0 + 1:k0 + 1 + n, :], THIRD, src3[:, k0:k0 + n, :],
            AL.mult, AL.add)
    if r_lo <= 0 < r_hi:
        nc.vector.tensor_scalar_mul(dst3[:, 0 - r_lo:1 - r_lo, :],
                                    src3[:, 0:1, :], EDGE)
    e = 2 * hin - 1
    if r_lo <= e < r_hi:
        nc.vector.tensor_scalar_mul(dst3[:, e - r_lo:e + 1 - r_lo, :],
                                    src3[:, hin - 1:hin, :], EDGE)


def _colblend(nc, src3, dst3, win):
    """Column-direction up2 blend: src3 (128, nr, win) -> dst3 (128, nr,
    2*win+2) cols [1, 2*win+1).  Cols 0 and 2*win+1 are pads (zeroed by
    caller).  Unnormalized by 1/0.75."""
    # even x=2l, l>=1 -> dst col 2l+1
    nc.vector.scalar_tensor_tensor(
        dst3[:, :, 3:3 + 2 * (win - 2) + 1:2],
        src3[:, :, 0:win - 1], THIRD, src3[:, :, 1:win],
        AL.mult, AL.add)
    # odd x=2l+1, l<=win-2 -> dst col 2l+2
    nc.vector.scalar_tensor_tensor(
        dst3[:, :, 2:2 + 2 * (win - 2) + 1:2],
        src3[:, :, 1:win], THIRD, src3[:, :, 0:win - 1],
        AL.mult, AL.add)
    nc.vector.tensor_scalar_mul(dst3[:, :, 1:2], src3[:, :, 0:1], EDGE)
    nc.vector.tensor_scalar_mul(dst3[:, :, 2 * win:2 * win + 1],
                                src3[:, :, win - 1:win], EDGE)


def _colblend3(nc, src3, dst3, win):
    """Merged (kc*rows) column blend: src3 (128, m, win) -> dst3 (128, m,
    2*win+2), single-instruction per phase."""
    nc.vector.scalar_tensor_tensor(
        dst3[:, :, 3:3 + 2 * (win - 2) + 1:2],
        src3[:, :, 0:win - 1], THIRD, src3[:, :, 1:win],
        AL.mult, AL.add)
    nc.vector.scalar_tensor_tensor(
        dst3[:, :, 2:2 + 2 * (win - 2) + 1:2],
        src3[:, :, 1:win], THIRD, src3[:, :, 0:win - 1],
        AL.mult, AL.add)
    nc.vector.tensor_scalar_mul(dst3[:, :, 1::2 * win - 1],
                                src3[:, :, 0::win - 1], EDGE)


def _memz(nc, ap):
    nc.vector.memset(ap.bitcast(F32), 0)


def build():
    nc = bacc.Bacc("TRN2", target_bir_lowering=False, debug=False,
                   num_devices=N_CORES)
    P = nc.declare_dram_parameter
    x_in = P("x_in", [SPC, 4, 128, 676], F32, isOutput=False)
    w1_in = P("w1_in", [4, 512, 9, 128], F32, isOutput=False)
    w2_in = P("w2_in", [2, 512, 9, 128], F32, isOutput=False)
    w3_in = P("w3_in", [2, 128, 256], F32, isOutput=False)
    txt9_in = P("txt9_in", [4, 128, 9, 256], F32, isOutput=False)
    txtl_in = P("txtl_in", [128, 4], F32, isOutput=False)
    tbd_in = P("tbd_in", [128, 2, 9], F32, isOutput=False)
    tbl_in = P("tbl_in", [1, 1], F32, isOutput=False)
    word_in = P("word_in", [12, 1024], F32, isOutput=False)
    score_in = P("score_in", [12, 2], F32, isOutput=False)
    t1_in = P("t1_in", [128, 4], F32, isOutput=False)
    t2_in = P("t2_in", [128, 2], F32, isOutput=False)
    b3_in = P("b3_in", [128, 2], F32, isOutput=False)
    out_d = P("out_d", [SPC, 104, 104], F32, isOutput=True)

    with tile.TileContext(nc) as tc:
        with (
            tc.tile_pool(name="sb", bufs=1) as sb,
            tc.tile_pool(name="ps", bufs=1, space="PSUM") as ps,
        ):
            r32 = F32R

            # ---------- small constant loads ----------
            word_sb = sb.tile([12, 1024], F32, tag="word")
            nc.sync.dma_start(word_sb[:], word_in[:, :])
            score_sb = sb.tile([12, 2], F32, tag="score")
            nc.sync.dma_start(score_sb[:], score_in[:, :])
            ones12 = sb.tile([12, 128], F32, tag="ones")
            nc.vector.memset(ones12[:], 1.0)
            txtl_sb = sb.tile([128, 4], F32, tag="txtl")
            nc.sync.dma_start(txtl_sb[:], txtl_in[:, :])
            tbd_sb = sb.tile([128, 2, 9], F32, tag="tbd")
            nc.sync.dma_start(tbd_sb[:], bass.AP(tbd_in, 0, [[18, 128], [9, 2], [1, 9]]))
            tbl_sb = sb.tile([1, 1], F32, tag="tbl")
            nc.sync.dma_start(tbl_sb[:], tbl_in[:, :])
            t1_sb = sb.tile([128, 4], F32, tag="t1")
            nc.sync.dma_start(t1_sb[:], t1_in[:, :])
            t2_sb = sb.tile([128, 2], F32, tag="t2")
            nc.sync.dma_start(t2_sb[:], t2_in[:, :])
            b3_sb = sb.tile([128, 2], F32, tag="b3")
            nc.sync.dma_start(b3_sb[:], b3_in[:, :])
            w3_sb = sb.tile([128, 2, 256], r32, tag="w3")
            nc.sync.dma_start(w3_sb[:], bass.AP(
                w3_in, 0, [[256, 128], [128 * 256, 2], [1, 256]]).bitcast(r32))

            beta_sb = sb.tile([1, 2], F32, tag="beta")
            s_bb = sb.tile([128, 2], F32, tag="sbb")
            wvT_sb = sb.tile([128, 8], F32, tag="wvt")

            # ---------- P0: text path -> F_dyn (c,t layout) + beta ----------
            txt9_sb = sb.tile([128, 4, 9, 256], F32, tag="wslab")
            nc.sync.dma_start(txt9_sb[:], bass.AP(
                txt9_in, 0,
                [[9 * 256, 128], [128 * 9 * 256, 4], [256, 9], [1, 256]]))

            wvps = ps.tile([128, 8], F32, tag="p0", bufs=3)
            for s in range(SPC):
                for kc in range(4):
                    i = s * 4 + kc
                    nc.tensor.matmul(
                        wvps[:, i:i + 1],
                        word_sb[:, s * 512 + kc * 128: s * 512 + (kc + 1) * 128],
                        score_sb[:, s:s + 1], start=True, stop=True)
            nc.vector.tensor_copy(wvT_sb[:], wvps[:])
            sbps = ps.tile([128, 2], F32, tag="p0", bufs=3)
            nc.tensor.matmul(sbps[:], ones12[:], score_sb[:], start=True, stop=True)
            nc.vector.tensor_copy(s_bb[:], sbps[:])

            f_dyn = []
            for s in range(SPC):
                fps = ps.tile([128, 2, 9], F32, tag="p0", bufs=3)
                for mc2 in range(2):
                    for t in range(9):
                        for kc in range(4):
                            nc.tensor.matmul(
                                fps[:, mc2, t:t + 1],
                                txt9_sb[:, kc, t, mc2 * 128:(mc2 + 1) * 128],
                                wvT_sb[:, s * 4 + kc:s * 4 + kc + 1],
                                start=(kc == 0), stop=(kc == 3))
                fd = sb.tile([128, 2, 9], r32, tag="fdyn", bufs=2)
                nc.vector.scalar_tensor_tensor(
                    fd[:], tbd_sb[:], s_bb[:, s:s + 1], fps[:], AL.mult, AL.add)
                f_dyn.append(fd)
                bps = ps.tile([1, 1], F32, tag="dyn", bufs=2)
                for kc in range(4):
                    nc.tensor.matmul(
                        bps[:], txtl_sb[:, kc:kc + 1],
                        wvT_sb[:, s * 4 + kc:s * 4 + kc + 1],
                        start=(kc == 0), stop=(kc == 3))
                nc.vector.scalar_tensor_tensor(
                    beta_sb[:, s:s + 1], tbl_sb[:], s_bb[0:1, s:s + 1], bps[:],
                    AL.mult, AL.add)

            # ---------- per-sample main pipeline ----------
            for s in range(SPC):
                # P1: load x, row-blend to xr_full (52 rows, width 26)
                x_sb = sb.tile([128, 4, 26, 26], r32, tag="x")
                nc.sync.dma_start(x_sb[:], bass.AP(
                    x_in, s * 4 * 128 * 676,
                    [[676, 128], [128 * 676, 4], [26, 26], [1, 26]]).bitcast(r32))
                xr = sb.tile([128, 4, 52, 26], r32, tag="xr")
                for kc in range(4):
                    _rowblend(nc, x_sb[:, kc], xr[:, kc], 0, 52, 26)

                # P2: conv1 (512->512), mc-outer with streamed weight slabs
                h1 = sb.tile([128, 4, 52, 52], r32, tag="h1")
                hb_pp = []
                for i in range(2):
                    hb_t = sb.tile([128, 4, 11, 54], r32, tag=f"ub2_{i}")
                    hb_pp.append(hb_t)
                for i in range(2):
                    _memz(nc, hb_pp[i][:, :, :, 0:1])
                    _memz(nc, hb_pp[i][:, :, :, 53:54])
                for mc in range(4):
                    w1s = sb.tile([128, 4, 9, 128], r32, tag="wslab")
                    nc.sync.dma_start(w1s[:], bass.AP(
                        w1_in, mc * 512 * 9 * 128,
                        [[9 * 128, 128], [128 * 9 * 128, 4], [128, 9], [1, 128]]
                    ).bitcast(r32))
                    for (y0, R) in BLOCKS1:
                        hb = sb.tile([128, 4, 11, 54], r32, tag="ubank", bufs=2)
                        # h0p rows [y0, y0+R+2) ; up2 rows r = h0p_row - 1
                        r_lo = max(0, y0 - 1)
                        r_hi = min(52, y0 + R + 1)
                        s_lo = r_lo - (y0 - 1)
                        s_hi = r_hi - (y0 - 1)
                        _memz(nc, hb[:, :, :R + 2, 0:1])
                        _memz(nc, hb[:, :, :R + 2, 53:54])
                        if s_lo > 0:
                            _memz(nc, hb[:, :, 0:s_lo, 1:53])
                        if s_hi < R + 2:
                            _memz(nc, hb[:, :, s_hi:R + 2, 1:53])
                        for kc in range(4):
                            _colblend(nc, xr[:, kc, r_lo:r_hi, :],
                                      hb[:, kc, s_lo:s_hi, :], 26)
                        ps1 = ps.tile([128, 9, 52], F32, tag="mm", bufs=3)
                        first = True
                        for t in range(9):
                            ky, kx = t // 3, t % 3
                            for kc in range(4):
                                nc.tensor.matmul(
                                    ps1[:, 0:R, :], w1s[:, kc, t, :],
                                    hb[:, kc, ky:ky + R, kx:kx + 52],
                                    start=first, stop=(t == 8 and kc == 3))
                                first = False
                        nc.scalar.activation(
                            h1[:, mc, y0:y0 + R, :], ps1[:, 0:R, :], AF.Relu,
                            bias=t1_sb[:, mc:mc + 1], scale=1.0)

                # P3+P4: conv2 + conv3 + dynamic conv, fused band loop
                w2f = sb.tile([128, 2, 4, 9, 128], r32, tag="wslab")
                for mc in range(2):
                    nc.sync.dma_start(w2f[:, mc], bass.AP(
                        w2_in, mc * 512 * 9 * 128,
                        [[9 * 128, 128], [128 * 9 * 128, 4], [128, 9], [1, 128]]
                    ).bitcast(r32))

                h2_pp = []
                for i in range(2):
                    h2_t = sb.tile([128, 24, 106], r32, tag=f"ub2_{i}")
                    h2_pp.append(h2_t)
                for i in range(2):
                    _memz(nc, h2_pp[i][:, :, 0:1])
                    _memz(nc, h2_pp[i][:, :, 105:106])
                t4 = {}

                t4_pp = []
                for i in range(4):
                    t4_t = sb.tile([128, 2, 6, 106], r32, tag=f"h4w{i}")
                    t4_pp.append(t4_t)
                for i in range(4):
                    _memz(nc, t4_pp[i][:, :, :, 0:1])
                    _memz(nc, t4_pp[i][:, :, :, 105:106])

                def new_t4(b):
                    tl = t4_pp[b % 4]
                    if b == 0:
                        _memz(nc, tl[:, :, 0:1, 1:105])
                    if b == NB2 - 1:
                        _memz(nc, tl[:, :, 5:6, 1:105])
                    t4[b] = tl
                    return tl

                def dyn_block(blk):
                    tl = t4.pop(blk)
                    psd = ps.tile([1, 4, 104], F32, tag="dyn", bufs=2)
                    first = True
                    for t in range(9):
                        ky, kx = t // 3, t % 3
                        for kc in range(2):
                            nc.tensor.matmul(
                                psd[:], f_dyn[s][:, kc, t:t + 1],
                                tl[:, kc, ky:ky + 4, kx:kx + 104],
                                start=first, stop=(t == 8 and kc == 1))
                            first = False
                    osb = sb.tile([1, 4, 104], F32, tag="outsb", bufs=2)
                    nc.scalar.activation(osb[:], psd[:], AF.Identity,
                                         bias=beta_sb[0:1, s:s + 1])
                    nc.sync.dma_start(
                        bass.AP(out_d, s * 10816 + blk * 416,
                                [[416, 1], [104, 4], [1, 104]]),
                        osb[:])

                new_t4(0)
                for b in range(NB2):
                    if b + 1 < NB2:
                        new_t4(b + 1)
                    # h2 band: rows [4b-1, 4b+5)
                    h2b = h2_pp[b % 2]
                    rb_lo = 4 * b - 1
                    r_lo = max(0, rb_lo)
                    r_hi = min(104, rb_lo + 6)
                    s_lo = r_lo - rb_lo
                    s_hi = r_hi - rb_lo
                    if s_lo > 0:
                        for kc in range(4):
                            _memz(nc, h2b[:, kc * 6:kc * 6 + s_lo, 1:105])
                    if s_hi < 6:
                        for kc in range(4):
                            _memz(nc, h2b[:, kc * 6 + s_hi:kc * 6 + 6, 1:105])
                    h2r = sb.tile([128, 24, 52], r32, tag="ublend", bufs=2)
                    for kc in range(4):
                        _rowblend(nc, h1[:, kc],
                                  h2r[:, kc * 6 + s_lo:kc * 6 + s_hi, :],
                                  r_lo, r_hi, 52)
                    if s_lo == 0 and s_hi == 6:
                        _colblend3(nc, h2r[:], h2b[:], 52)
                    else:
                        for kc in range(4):
                            _colblend(nc, h2r[:, kc * 6 + s_lo:kc * 6 + s_hi, :],
                                      h2b[:, kc * 6 + s_lo:kc * 6 + s_hi, :], 52)
                    # conv2 -> h3 band (relu)
                    h3b = sb.tile([128, 2, 4, 104], r32, tag="h3band", bufs=2)
                    for mc in range(2):
                        ps2 = ps.tile([128, 4, 104], F32, tag="mm", bufs=3)
                        first = True
                        for t in range(9):
                            ky, kx = t // 3, t % 3
                            for kc in range(4):
                                nc.tensor.matmul(
                                    ps2[:], w2f[:, mc, kc, t, :],
                                    h2b[:, kc * 6 + ky:kc * 6 + ky + 4, kx:kx + 104],
                                    start=first, stop=(t == 8 and kc == 3))
                                first = False
                        nc.scalar.activation(h3b[:, mc], ps2[:], AF.Relu,
                                             bias=t2_sb[:, mc:mc + 1], scale=1.0)
                    # conv3 1x1 -> h4 sliding window tiles
                    for mc3 in range(2):
                        ps3 = ps.tile([128, 4, 104], F32, tag="mm", bufs=3)
                        nc.tensor.matmul(ps3[:], w3_sb[:, 0, mc3 * 128:(mc3 + 1) * 128],
                                         h3b[:, 0], start=True, stop=False)
                        nc.tensor.matmul(ps3[:], w3_sb[:, 1, mc3 * 128:(mc3 + 1) * 128],
                                         h3b[:, 1], start=False, stop=True)
                        nc.scalar.activation(t4[b][:, mc3, 1:5, 1:105], ps3[:],
                                             AF.Identity, bias=b3_sb[:, mc3:mc3 + 1])
                        if b > 0:
                            nc.scalar.activation(
                                t4[b - 1][:, mc3, 5:6, 1:105], ps3[:, 0:1, :],
                                AF.Identity, bias=b3_sb[:, mc3:mc3 + 1])
                        if b + 1 < NB2:
                            nc.scalar.activation(
                                t4[b + 1][:, mc3, 0:1, 1:105], ps3[:, 3:4, :],
                                AF.Identity, bias=b3_sb[:, mc3:mc3 + 1])
                    if b >= 1:
                        dyn_block(b - 1)
                dyn_block(NB2 - 1)
    nc.compile()
    return nc


def _prep_host(inputs):
    """Fold BN + up2 scale into weights, build per-core input maps."""
    f = np.float32
    s1 = (inputs["bn1_g"] / np.sqrt(inputs["bn1_v"] + EPS)).astype(f)
    s2 = (inputs["bn2_g"] / np.sqrt(inputs["bn2_v"] + EPS)).astype(f)
    w1f = (inputs["conv1_w"] * (s1 * 0.5625)[:, None, None, None]).astype(f)
    w2f = (inputs["conv2_w"] * (s2 * 0.5625)[:, None, None, None]).astype(f)
    t1 = (inputs["bn1_b"] - inputs["bn1_m"] * s1).astype(f)
    t2 = (inputs["bn2_b"] - inputs["bn2_m"] * s2).astype(f)

    # lhsT layouts
    w1_h = np.ascontiguousarray(
        w1f.reshape(4, 128, 512, 9).transpose(0, 2, 3, 1))  # (mc, ci, t, co)
    w2_h = np.ascontiguousarray(
        w2f.reshape(2, 128, 512, 9).transpose(0, 2, 3, 1))
    w3_h = np.ascontiguousarray(
        inputs["conv3_w"][:, :, 0, 0].T.reshape(2, 128, 256)).astype(f)
    txt_w = inputs["txt_w"].astype(f)
    txt9_h = np.ascontiguousarray(
        txt_w[:2304].reshape(256, 9, 512).transpose(2, 1, 0)
        .reshape(4, 128, 9, 256))
    txtl_h = np.ascontiguousarray(txt_w[2304].reshape(4, 128).T)
    txt_b = inputs["txt_b"].astype(f)
    tbd_h = np.ascontiguousarray(
        txt_b[:2304].reshape(256, 9).reshape(2, 128, 9).transpose(1, 0, 2))
    tbl_h = np.array([[txt_b[2304]]], f)
    t1_h = np.ascontiguousarray(t1.reshape(4, 128).T)
    t2_h = np.ascontiguousarray(t2.reshape(2, 128).T)
    b3_h = np.ascontiguousarray(inputs["conv3_b"].astype(f).reshape(2, 128).T)

    shared = dict(w1_in=w1_h, w2_in=w2_h, w3_in=w3_h, txt9_in=txt9_h,
                  txtl_in=txtl_h, tbd_in=tbd_h, tbl_in=tbl_h,
                  t1_in=t1_h, t2_in=t2_h, b3_in=b3_h)

    x = inputs["x"].astype(f)
    word = inputs["word"].astype(f)
    score = inputs["score"].astype(f)
    in_maps = []
    for c in range(N_CORES):
        g0 = c * SPC
        m = dict(shared)
        m["x_in"] = np.ascontiguousarray(
            x[g0:g0 + SPC].reshape(SPC, 4, 128, 676))
        m["word_in"] = np.ascontiguousarray(
            word[:, g0:g0 + SPC, :].transpose(0, 1, 2).reshape(12, SPC * 512))
        m["score_in"] = np.ascontiguousarray(score[:, g0:g0 + SPC, 0])
        in_maps.append(m)
    return in_maps


def kernel(**inputs) -> np.ndarray:
    if "nc" not in _CACHE:
        _CACHE["nc"] = build()
    nc = _CACHE["nc"]
    in_maps = _prep_host(inputs)
    import time
    t0 = time.time()
    res = run_bass_kernel_spmd(nc, in_maps, list(range(N_CORES)))
    _CACHE["last_run_seconds"] = time.time() - t0
    out = np.concatenate([res.results[c]["out_d"] for c in range(N_CORES)], 0)
    return out.reshape(16, 1, 104, 104).astype(np.float32)


if __name__ == "__main__":
    import time
    t0 = time.time()
    nc = build()
    print(f"build+bacc-compile OK in {time.time()-t0:.1f}s", flush=True)
